# revision 3
# baseline (speedup 1.0000x reference)
"""ESM2 contact predictor head on 8 Trainium2 NeuronCores.

Computes out[b, i, j] = sigmoid(x[b,i] @ W @ x[b,j] + bias) for
x: (8, 2050, 320) f32, W: (320, 320) f32, bias: (1,) f32.

Sharding: data-parallel over batch — core c handles batch element c.

Per-core algorithm (all matmuls in float32r at full PE rate):
  host:  xt slabs of x[c].T with D=320 split as 128+128+64; the 64-row
         slab is DUPLICATED into both partition halves so slab-2 matmuls
         run as K=64 row-packed pairs (two concurrent matmuls in
         disjoint 64-row groups of the PE array).
         wp = W with rows/cols 256:320 duplicated to 320:384 likewise.
  chip:  u = wp.T @ xt                  == (x[c] @ W).T   (with dup slab)
         for each 128-row strip i of the (2050, 2050) output:
             logits[i, j] = sum_k u[k, i] * xt[k, j]   (PE, f32 PSUM)
             out_strip = sigmoid(logits + bias)        (ScalarE from PSUM)
             DMA strip -> HBM
"""

import numpy as np

import concourse.bass as bass
import concourse.mybir as mybir
import concourse.tile as tile
from concourse import bacc
from concourse.bass_utils import run_bass_kernel_spmd

N_CORES = 8
B, L, D = 8, 2050, 320
KT = 3            # K slabs: 128, 128, 64(duplicated)
F32 = mybir.dt.float32
F32R = mybir.dt.float32r
SIG = mybir.ActivationFunctionType.Sigmoid

J_TAIL = 2048
CHUNK = 1024      # input DMA chunk (columns)

# i-dim strips: 2-row tail strip FIRST (hides in pipeline fill), then 16 full
I_STRIPS = [(2048, 2)] + [(s * 128, 128) for s in range(16)]

_cache = {}


def _build(bias_val: float):
    nc = bacc.Bacc("TRN2", target_bir_lowering=False, debug=False,
                   num_devices=N_CORES)
    xt_main_d = nc.dram_tensor("xt_main", [2, 128, KT, CHUNK], F32R,
                               kind="ExternalInput")
    xt_tail_d = nc.dram_tensor("xt_tail", [128, KT, 2], F32R,
                               kind="ExternalInput")
    w_d = nc.dram_tensor("w", [384, 384], F32R, kind="ExternalInput")
    out_d = nc.dram_tensor("out", [L, L], F32, kind="ExternalOutput")

    w_r = w_d.ap().rearrange("(k p) e -> p k e", p=128)     # (128, 3, 384)

    with tile.TileContext(nc) as tc:
        with (
            tc.tile_pool(name="persist", bufs=1) as pp,
            tc.tile_pool(name="outp", bufs=3) as outp,
            tc.tile_pool(name="psum", bufs=2, space="PSUM") as psp,
        ):
            bias_t = pp.tile([128, 1], F32)
            nc.vector.memset(bias_t[:], bias_val)

            w_sb = pp.tile([128, KT, 384], F32R)
            nc.sync.dma_start(w_sb[:], w_r)

            xt_sb = pp.tile([128, KT, L], F32R)
            nc.sync.dma_start(xt_sb[:, :, 0:CHUNK], xt_main_d.ap()[0])
            nc.sync.dma_start(xt_sb[:, :, J_TAIL:L], xt_tail_d.ap())
            nc.sync.dma_start(xt_sb[:, :, CHUNK:2 * CHUNK], xt_main_d.ap()[1])

            u_sb = pp.tile([128, KT, L], F32R)

            def mm_group(psA, psB, lhs, a0, b0, nsz):
                """Two accumulation groups over the 3 K slabs; the two K=64
                slab-2 matmuls are adjacent and row-packed (rows 0-63 /
                64-127) so they stream concurrently."""
                nc.tensor.matmul(psA, lhsT=lhs(0), rhs=xt_sb[:, 0, a0:a0 + nsz],
                                 start=True, stop=False)
                nc.tensor.matmul(psA, lhsT=lhs(1), rhs=xt_sb[:, 1, a0:a0 + nsz],
                                 start=False, stop=False)
                nc.tensor.matmul(psB, lhsT=lhs(0), rhs=xt_sb[:, 0, b0:b0 + nsz],
                                 start=True, stop=False)
                nc.tensor.matmul(psB, lhsT=lhs(1), rhs=xt_sb[:, 1, b0:b0 + nsz],
                                 start=False, stop=False)
                lo, hi = lhs(2)
                nc.tensor.matmul(psA, lhsT=lo, rhs=xt_sb[0:64, 2, a0:a0 + nsz],
                                 start=False, stop=True)
                nc.tensor.matmul(psB, lhsT=hi, rhs=xt_sb[64:128, 2, b0:b0 + nsz],
                                 start=False, stop=True)

            def mm_tail(ps, lhs):
                """Single accumulation group for the 2 tail columns."""
                nc.tensor.matmul(ps, lhsT=lhs(0), rhs=xt_sb[:, 0, J_TAIL:L],
                                 start=True, stop=False)
                nc.tensor.matmul(ps, lhsT=lhs(1), rhs=xt_sb[:, 1, J_TAIL:L],
                                 start=False, stop=False)
                lo, _ = lhs(2)
                nc.tensor.matmul(ps, lhsT=lo, rhs=xt_sb[0:64, 2, J_TAIL:L],
                                 start=False, stop=True)

            # ---- phase 1: u = wp.T @ xt  (u[e, i], e on partitions) ----
            for et in range(KT):
                e0 = et * 128

                def wlhs(k, e0=e0):
                    if k == 2:
                        return (w_sb[0:64, 2, e0:e0 + 128],
                                w_sb[64:128, 2, e0:e0 + 128])
                    return w_sb[:, k, e0:e0 + 128]

                for half in range(2):
                    a0 = half * 1024
                    psA = psp.tile([128, 512], F32, tag="small", bufs=2,
                                   name="psA")
                    psB = psp.tile([128, 512], F32, tag="small", bufs=2,
                                   name="psB")
                    mm_group(psA[:, :], psB[:, :], wlhs, a0, a0 + 512, 512)
                    nc.vector.tensor_copy(u_sb[:, et, a0:a0 + 512], psA[:, :])
                    nc.vector.tensor_copy(u_sb[:, et, a0 + 512:a0 + 1024],
                                          psB[:, :])
                psT = psp.tile([128, 512], F32, tag="small", bufs=2, name="psT")
                mm_tail(psT[:, :2], wlhs)
                nc.vector.tensor_copy(u_sb[:, et, J_TAIL:L], psT[:, :2])

            # ---- phase 2: logits strips -> sigmoid -> DMA out ----
            for (i0, isz) in I_STRIPS:
                def ulhs(k, i0=i0, isz=isz):
                    if k == 2:
                        return (u_sb[0:64, 2, i0:i0 + isz],
                                u_sb[64:128, 2, i0:i0 + isz])
                    return u_sb[:, k, i0:i0 + isz]

                strip = outp.tile([128, L], F32, tag="strip", bufs=3,
                                  name="strip")
                for jp in range(2):
                    a0 = jp * 1024
                    ps = psp.tile([128, 1024], F32, tag="pair", bufs=3,
                                  name="ps")
                    mm_group(ps[:isz, 0:512], ps[:isz, 512:1024], ulhs,
                             a0, a0 + 512, 512)
                    nc.scalar.activation(
                        strip[:isz, a0:a0 + 1024], ps[:isz, :], SIG,
                        bias=bias_t[:isz, :],
                    )
                pst = psp.tile([128, 512], F32, tag="small", bufs=2, name="pst")
                mm_tail(pst[:isz, :2], ulhs)
                nc.scalar.activation(
                    strip[:isz, J_TAIL:L], pst[:isz, :2], SIG,
                    bias=bias_t[:isz, :],
                )
                nc.sync.dma_start(out_d.ap()[i0:i0 + isz, :], strip[:isz, :])

    nc.compile()
    return nc


last_results = None


def _host_pack(x, W):
    xT = x.transpose(0, 2, 1)  # (B, 320, 2050)
    full = np.empty((B, 128, KT, L), np.float32)
    full[:, :, 0, :] = xT[:, 0:128]
    full[:, :, 1, :] = xT[:, 128:256]
    full[:, 0:64, 2, :] = xT[:, 256:320]
    full[:, 64:128, 2, :] = xT[:, 256:320]
    xt_main = np.ascontiguousarray(
        full[..., :J_TAIL].reshape(B, 128, KT, 2, CHUNK)
        .transpose(0, 3, 1, 2, 4))
    xt_tail = np.ascontiguousarray(full[..., J_TAIL:L])
    wp = np.empty((384, 384), np.float32)
    wp[0:320, 0:320] = W
    wp[320:384, 0:320] = W[256:320, :]
    wp[:, 320:384] = wp[:, 256:320]
    return xt_main, xt_tail, wp


def kernel(x, W, b, _trace=False):
    global last_results
    x = np.ascontiguousarray(np.asarray(x, dtype=np.float32))
    W = np.asarray(W, dtype=np.float32)
    b = np.asarray(b, dtype=np.float32)
    bias_val = float(b[0])

    if bias_val not in _cache:
        _cache.clear()
        _cache[bias_val] = _build(bias_val)
    nc = _cache[bias_val]

    xt_main, xt_tail, wp = _host_pack(x, W)
    in_maps = [{"xt_main": xt_main[c], "xt_tail": xt_tail[c], "w": wp}
               for c in range(N_CORES)]
    res = run_bass_kernel_spmd(nc, in_maps, core_ids=list(range(N_CORES)),
                               trace=_trace)
    last_results = res
    out = np.stack([res.results[c]["out"] for c in range(N_CORES)], axis=0)
    return out.astype(np.float32, copy=False)


# revision 4
# speedup vs baseline: 1.1161x; 1.1161x over previous
"""ESM2 contact predictor head on 8 Trainium2 NeuronCores.

Computes out[b, i, j] = sigmoid(x[b,i] @ W @ x[b,j] + bias) for
x: (8, 2050, 320) f32, W: (320, 320) f32, bias: (1,) f32.

Sharding: data-parallel over batch — core c handles batch element c.

Per-core algorithm (all matmuls in float32r at full PE rate):
  host:  xt slabs of x[c].T with D=320 split as 128+128+64; the 64-row
         slab is DUPLICATED into both partition halves so slab-2 matmuls
         run as K=64 row-packed pairs (two concurrent matmuls in
         disjoint 64-row groups of the PE array).
         wp = W with rows/cols 256:320 duplicated to 320:384 likewise.
  chip:  u = wp.T @ xt                  == (x[c] @ W).T   (with dup slab)
         for each 128-row strip i of the (2050, 2050) output:
             logits[i, j] = sum_k u[k, i] * xt[k, j]   (PE, f32 PSUM)
             out_strip = sigmoid(logits + bias)        (ScalarE from PSUM)
             DMA strip -> HBM
"""

import numpy as np

import concourse.bass as bass
import concourse.mybir as mybir
import concourse.tile as tile
from concourse import bacc
from concourse.bass_utils import run_bass_kernel_spmd

N_CORES = 8
B, L, D = 8, 2050, 320
KT = 3            # K slabs: 128, 128, 64(duplicated)
F32 = mybir.dt.float32
F32R = mybir.dt.float32r
SIG = mybir.ActivationFunctionType.Sigmoid

J_TAIL = 2048
CHUNK = 1024      # input DMA chunk (columns)

# i-dim strips: 2-row tail strip FIRST (hides in pipeline fill), then 16 full
I_STRIPS = [(2048, 2)] + [(s * 128, 128) for s in range(16)]

_cache = {}


def _build(bias_val: float):
    nc = bacc.Bacc("TRN2", target_bir_lowering=False, debug=False,
                   num_devices=N_CORES)
    xt_main_d = nc.dram_tensor("xt_main", [2, 128, KT, CHUNK], F32R,
                               kind="ExternalInput")
    xt_tail_d = nc.dram_tensor("xt_tail", [128, KT, 2], F32R,
                               kind="ExternalInput")
    w_d = nc.dram_tensor("w", [384, 384], F32R, kind="ExternalInput")
    out_d = nc.dram_tensor("out", [L, L], F32, kind="ExternalOutput")

    w_r = w_d.ap().rearrange("(k p) e -> p k e", p=128)     # (128, 3, 384)

    with tile.TileContext(nc) as tc:
        with (
            tc.tile_pool(name="persist", bufs=1) as pp,
            tc.tile_pool(name="outp", bufs=3) as outp,
            tc.tile_pool(name="psum", bufs=2, space="PSUM") as psp,
        ):
            bias_t = pp.tile([128, 1], F32)
            nc.vector.memset(bias_t[:], bias_val)

            w_sb = pp.tile([128, KT, 384], F32R)
            nc.sync.dma_start(w_sb[:], w_r)

            xt_sb = pp.tile([128, KT, L], F32R)
            nc.sync.dma_start(xt_sb[:, :, 0:CHUNK], xt_main_d.ap()[0])
            nc.sync.dma_start(xt_sb[:, :, J_TAIL:L], xt_tail_d.ap())
            nc.sync.dma_start(xt_sb[:, :, CHUNK:2 * CHUNK], xt_main_d.ap()[1])

            u_sb = pp.tile([128, KT, L], F32R)

            # PE warmup: dummy matmuls during the input-DMA window so the
            # HAM clock-gate is released before real work starts.
            warm_sb = pp.tile([128, 128], F32R)
            nc.vector.memset(warm_sb.bitcast(F32)[:], 1.0)
            for wi in range(12):
                psw = psp.tile([128, 512], F32, tag="small", bufs=2,
                               name="psw")
                nc.tensor.matmul(psw[:, :128], lhsT=warm_sb[:],
                                 rhs=warm_sb[:], start=True, stop=True)

            def mm_acc(ps, lhs, j0, nsz):
                """One accumulation group over the 3 K slabs (slab 2
                zero-padded to K=128)."""
                for k in range(KT):
                    nc.tensor.matmul(ps, lhsT=lhs(k),
                                     rhs=xt_sb[:, k, j0:j0 + nsz],
                                     start=(k == 0), stop=(k == KT - 1))

            # ---- phase 1: u = wp.T @ xt  (u[e, i], e on partitions) ----
            for et in range(KT):
                e0 = et * 128

                def wlhs(k, e0=e0):
                    return w_sb[:, k, e0:e0 + 128]

                for nt in range(4):
                    n0 = nt * 512
                    ps1 = psp.tile([128, 512], F32, tag="small", bufs=2,
                                   name="ps1")
                    mm_acc(ps1[:, :], wlhs, n0, 512)
                    nc.vector.tensor_copy(u_sb[:, et, n0:n0 + 512], ps1[:, :])
                psT = psp.tile([128, 512], F32, tag="small", bufs=2, name="psT")
                mm_acc(psT[:, :2], wlhs, J_TAIL, 2)
                nc.vector.tensor_copy(u_sb[:, et, J_TAIL:L], psT[:, :2])

            # ---- phase 2: logits strips -> sigmoid -> DMA out ----
            for (i0, isz) in I_STRIPS:
                def ulhs(k, i0=i0, isz=isz):
                    return u_sb[:, k, i0:i0 + isz]

                strip = outp.tile([128, L], F32, tag="strip", bufs=3,
                                  name="strip")
                for jp in range(2):
                    a0 = jp * 1024
                    ps = psp.tile([128, 1024], F32, tag="pair", bufs=3,
                                  name="ps")
                    mm_acc(ps[:isz, 0:512], ulhs, a0, 512)
                    mm_acc(ps[:isz, 512:1024], ulhs, a0 + 512, 512)
                    nc.scalar.activation(
                        strip[:isz, a0:a0 + 1024], ps[:isz, :], SIG,
                        bias=bias_t[:isz, :],
                    )
                pst = psp.tile([128, 512], F32, tag="small", bufs=2, name="pst")
                mm_acc(pst[:isz, :2], ulhs, J_TAIL, 2)
                nc.scalar.activation(
                    strip[:isz, J_TAIL:L], pst[:isz, :2], SIG,
                    bias=bias_t[:isz, :],
                )
                nc.sync.dma_start(out_d.ap()[i0:i0 + isz, :], strip[:isz, :])

    nc.compile()
    return nc


last_results = None


def _host_pack(x, W):
    xT = x.transpose(0, 2, 1)  # (B, 320, 2050)
    full = np.empty((B, 128, KT, L), np.float32)
    full[:, :, 0, :] = xT[:, 0:128]
    full[:, :, 1, :] = xT[:, 128:256]
    full[:, 0:64, 2, :] = xT[:, 256:320]
    full[:, 64:128, 2, :] = 0.0
    xt_main = np.ascontiguousarray(
        full[..., :J_TAIL].reshape(B, 128, KT, 2, CHUNK)
        .transpose(0, 3, 1, 2, 4))
    xt_tail = np.ascontiguousarray(full[..., J_TAIL:L])
    wp = np.zeros((384, 384), np.float32)
    wp[0:320, 0:320] = W
    return xt_main, xt_tail, wp


def kernel(x, W, b, _trace=False):
    global last_results
    x = np.ascontiguousarray(np.asarray(x, dtype=np.float32))
    W = np.asarray(W, dtype=np.float32)
    b = np.asarray(b, dtype=np.float32)
    bias_val = float(b[0])

    if bias_val not in _cache:
        _cache.clear()
        _cache[bias_val] = _build(bias_val)
    nc = _cache[bias_val]

    xt_main, xt_tail, wp = _host_pack(x, W)
    in_maps = [{"xt_main": xt_main[c], "xt_tail": xt_tail[c], "w": wp}
               for c in range(N_CORES)]
    res = run_bass_kernel_spmd(nc, in_maps, core_ids=list(range(N_CORES)),
                               trace=_trace)
    last_results = res
    out = np.stack([res.results[c]["out"] for c in range(N_CORES)], axis=0)
    return out.astype(np.float32, copy=False)


# revision 5
# speedup vs baseline: 1.1850x; 1.0617x over previous
"""ESM2 contact predictor head on 8 Trainium2 NeuronCores.

Computes out[b, i, j] = sigmoid(x[b,i] @ W @ x[b,j] + bias) for
x: (8, 2050, 320) f32, W: (320, 320) f32, bias: (1,) f32.

Sharding: data-parallel over batch — core c handles batch element c.

Per-core algorithm (all matmuls in float32r at full PE rate):
  host:  xt slabs of x[c].T with D=320 split as 128+128+64; the 64-row
         slab is DUPLICATED into both partition halves so slab-2 matmuls
         run as K=64 row-packed pairs (two concurrent matmuls in
         disjoint 64-row groups of the PE array).
         wp = W with rows/cols 256:320 duplicated to 320:384 likewise.
  chip:  u = wp.T @ xt                  == (x[c] @ W).T   (with dup slab)
         for each 128-row strip i of the (2050, 2050) output:
             logits[i, j] = sum_k u[k, i] * xt[k, j]   (PE, f32 PSUM)
             out_strip = sigmoid(logits + bias)        (ScalarE from PSUM)
             DMA strip -> HBM
"""

import numpy as np

import concourse.bass as bass
import concourse.mybir as mybir
import concourse.tile as tile
from concourse import bacc
from concourse.bass_utils import run_bass_kernel_spmd

N_CORES = 8
B, L, D = 8, 2050, 320
KT = 3            # K slabs: 128, 128, 64(duplicated)
F32 = mybir.dt.float32
F32R = mybir.dt.float32r
SIG = mybir.ActivationFunctionType.Sigmoid

J_TAIL = 2048
CHUNK = 512       # input DMA chunk (columns)

# i-dim strips: 2-row tail strip FIRST (hides in pipeline fill), then 16 full
I_STRIPS = [(2048, 2)] + [(s * 128, 128) for s in range(16)]

_cache = {}


def _build(bias_val: float):
    nc = bacc.Bacc("TRN2", target_bir_lowering=False, debug=False,
                   num_devices=N_CORES)
    xt_main_d = nc.dram_tensor("xt_main", [4, 128, KT, CHUNK], F32R,
                               kind="ExternalInput")
    xt_tail_d = nc.dram_tensor("xt_tail", [128, KT, 2], F32R,
                               kind="ExternalInput")
    w_d = nc.dram_tensor("w", [384, 384], F32R, kind="ExternalInput")
    out_d = nc.dram_tensor("out", [L, L], F32, kind="ExternalOutput")

    w_r = w_d.ap().rearrange("(k p) e -> p k e", p=128)     # (128, 3, 384)

    with tile.TileContext(nc) as tc:
        with (
            tc.tile_pool(name="persist", bufs=1) as pp,
            tc.tile_pool(name="outp", bufs=3) as outp,
            tc.tile_pool(name="psum", bufs=2, space="PSUM") as psp,
        ):
            bias_t = pp.tile([128, 1], F32)
            nc.vector.memset(bias_t[:], bias_val)

            w_sb = pp.tile([128, KT, 384], F32R)
            nc.sync.dma_start(w_sb[:], w_r)

            xt_sb = pp.tile([128, KT, L], F32R)
            nc.sync.dma_start(xt_sb[:, :, 0:CHUNK], xt_main_d.ap()[0])
            nc.sync.dma_start(xt_sb[:, :, CHUNK:2 * CHUNK], xt_main_d.ap()[1])
            nc.sync.dma_start(xt_sb[:, :, J_TAIL:L], xt_tail_d.ap())
            nc.sync.dma_start(xt_sb[:, :, 2 * CHUNK:3 * CHUNK], xt_main_d.ap()[2])
            nc.sync.dma_start(xt_sb[:, :, 3 * CHUNK:4 * CHUNK], xt_main_d.ap()[3])

            u_sb = pp.tile([128, KT, L], F32R)

            # PE warmup: dummy matmuls during the input-DMA window so the
            # HAM clock-gate is released before real work starts.
            warm_sb = pp.tile([128, 128], F32R)
            nc.vector.memset(warm_sb.bitcast(F32)[:], 1.0)
            for wi in range(12):
                psw = psp.tile([128, 512], F32, tag="small", bufs=2,
                               name="psw")
                nc.tensor.matmul(psw[:, :128], lhsT=warm_sb[:],
                                 rhs=warm_sb[:], start=True, stop=True)

            def mm_acc(ps, lhs, j0, nsz):
                """One accumulation group over the 3 K slabs (slab 2
                zero-padded to K=128)."""
                for k in range(KT):
                    nc.tensor.matmul(ps, lhsT=lhs(k),
                                     rhs=xt_sb[:, k, j0:j0 + nsz],
                                     start=(k == 0), stop=(k == KT - 1))

            # ---- phase 1: u = wp.T @ xt  (u[e, i], e on partitions) ----
            def wlhs_of(et):
                def wlhs(k, e0=et * 128):
                    return w_sb[:, k, e0:e0 + 128]
                return wlhs

            for nt in range(4):
                n0 = nt * 512
                for et in range(KT):
                    ps1 = psp.tile([128, 512], F32, tag="small", bufs=2,
                                   name="ps1")
                    mm_acc(ps1[:, :], wlhs_of(et), n0, 512)
                    nc.vector.tensor_copy(u_sb[:, et, n0:n0 + 512], ps1[:, :])
            for et in range(KT):
                psT = psp.tile([128, 512], F32, tag="small", bufs=2, name="psT")
                mm_acc(psT[:, :2], wlhs_of(et), J_TAIL, 2)
                nc.vector.tensor_copy(u_sb[:, et, J_TAIL:L], psT[:, :2])

            # ---- phase 2: logits strips -> sigmoid -> DMA out ----
            for (i0, isz) in I_STRIPS:
                def ulhs(k, i0=i0, isz=isz):
                    return u_sb[:, k, i0:i0 + isz]

                strip = outp.tile([128, L], F32, tag="strip", bufs=3,
                                  name="strip")
                for jp in range(2):
                    a0 = jp * 1024
                    ps = psp.tile([128, 1024], F32, tag="pair", bufs=3,
                                  name="ps")
                    for k in range(KT):
                        # consecutive matmuls share lhsT and alternate banks
                        nc.tensor.matmul(ps[:isz, 0:512], lhsT=ulhs(k),
                                         rhs=xt_sb[:, k, a0:a0 + 512],
                                         start=(k == 0), stop=(k == KT - 1))
                        nc.tensor.matmul(ps[:isz, 512:1024], lhsT=ulhs(k),
                                         rhs=xt_sb[:, k, a0 + 512:a0 + 1024],
                                         start=(k == 0), stop=(k == KT - 1))
                    nc.scalar.activation(
                        strip[:isz, a0:a0 + 1024], ps[:isz, :], SIG,
                        bias=bias_t[:isz, :],
                    )
                pst = psp.tile([128, 512], F32, tag="small", bufs=2, name="pst")
                for kk, k in enumerate((2, 1, 0)):
                    nc.tensor.matmul(pst[:isz, :2], lhsT=ulhs(k),
                                     rhs=xt_sb[:, k, J_TAIL:L],
                                     start=(kk == 0), stop=(kk == KT - 1))
                nc.scalar.activation(
                    strip[:isz, J_TAIL:L], pst[:isz, :2], SIG,
                    bias=bias_t[:isz, :],
                )
                nc.sync.dma_start(out_d.ap()[i0:i0 + isz, :], strip[:isz, :])

    nc.compile()
    return nc


last_results = None


def _host_pack(x, W):
    xT = x.transpose(0, 2, 1)  # (B, 320, 2050)
    full = np.empty((B, 128, KT, L), np.float32)
    full[:, :, 0, :] = xT[:, 0:128]
    full[:, :, 1, :] = xT[:, 128:256]
    full[:, 0:64, 2, :] = xT[:, 256:320]
    full[:, 64:128, 2, :] = 0.0
    xt_main = np.ascontiguousarray(
        full[..., :J_TAIL].reshape(B, 128, KT, 4, CHUNK)
        .transpose(0, 3, 1, 2, 4))
    xt_tail = np.ascontiguousarray(full[..., J_TAIL:L])
    wp = np.zeros((384, 384), np.float32)
    wp[0:320, 0:320] = W
    return xt_main, xt_tail, wp


def kernel(x, W, b, _trace=False):
    global last_results
    x = np.ascontiguousarray(np.asarray(x, dtype=np.float32))
    W = np.asarray(W, dtype=np.float32)
    b = np.asarray(b, dtype=np.float32)
    bias_val = float(b[0])

    if bias_val not in _cache:
        _cache.clear()
        _cache[bias_val] = _build(bias_val)
    nc = _cache[bias_val]

    xt_main, xt_tail, wp = _host_pack(x, W)
    in_maps = [{"xt_main": xt_main[c], "xt_tail": xt_tail[c], "w": wp}
               for c in range(N_CORES)]
    res = run_bass_kernel_spmd(nc, in_maps, core_ids=list(range(N_CORES)),
                               trace=_trace)
    last_results = res
    out = np.stack([res.results[c]["out"] for c in range(N_CORES)], axis=0)
    return out.astype(np.float32, copy=False)
